# revision 6
# baseline (speedup 1.0000x reference)
"""FFNN-Transducer joint lattice, layout B': t-major output, per-u PSUM tiles.

Per core:
  PE:   ep[j,t] = jw1enc.T @ encT (8 matmuls); per (u, t-block of 128) one
        joint matmul ps[t,:] = hid_u_chunk.T @ jw2 (lhsT = contiguous fp16
        128-col slice of hid_u -> FWL; jw2 streams N=88).
  ACT:  per u one instruction hid_u = tanh(ep + bias_u) over [128, 1024]
        (fused per-partition bias).
  DVE:  per u two PSUM->SBUF evacuations ([128,440] tb0-4 and [128,264] tb5-7)
        with fp32->fp16 cast, scatter-interleaved into a (tb, u, v) staging
        layout so each DMA descriptor is 4 u's x 88 v = 704B contiguous.
  DMA:  one store per 4 u's; out DRAM layout [t, u*88+v] fp16 (t padded 1024).
"""

import os
import sys

for _p in ("/opt/trn_rl_repo", "/root/.axon_site/_ro/trn_rl_repo"):
    if os.path.isdir(_p) and _p not in sys.path:
        sys.path.append(_p)

import numpy as np

import concourse.bass as bass
import concourse.tile as tile
from concourse import bacc, mybir
from concourse.bass_utils import run_bass_kernel_spmd

B, T, E = 8, 1000, 512
U = 100
U1 = U + 1
H, D, P = 2, 256, 256
J, V = 128, 88
BLANK = V - 1
N_CORES = 8

TP = 1024           # padded T
TB = 128            # t per joint matmul (lhsT cols)
NTB = TP // TB      # 8
TBA = 5             # t-blocks in PSUM tile a (5*88*4B = 1760 <= 2048)
UD = 4              # u's per output DMA

F32 = mybir.dt.float32
F16 = mybir.dt.float16

_CACHE = {}


def _build_program(reps=1):
    nc = bacc.Bacc("TRN2", target_bir_lowering=False, debug=False)

    encT = nc.dram_tensor("encT", [E, TP], F16, kind="ExternalInput").ap()
    jw1enc = nc.dram_tensor("jw1enc", [E, J], F16, kind="ExternalInput").ap()
    jw2d = nc.dram_tensor("jw2d", [J, V], F16, kind="ExternalInput").ap()
    biasT = nc.dram_tensor("biasT", [J, U1], F32, kind="ExternalInput").ap()
    out = nc.dram_tensor("out", [TP, U1 * V], F16, kind="ExternalOutput").ap()

    with tile.TileContext(nc) as tc:
        with (
            tc.tile_pool(name="singles", bufs=1) as singles,
            tc.tile_pool(name="hidp", bufs=3) as hidp,
            tc.tile_pool(name="stgp", bufs=3) as stgp,
            tc.tile_pool(name="psp", bufs=8, space="PSUM") as psp,
        ):
            encT_sb = []
            for k in range(4):
                t_ = singles.tile([128, TP], F16, tag=f"encT{k}")
                nc.sync.dma_start(out=t_[:, :], in_=encT[k * 128:(k + 1) * 128, :])
                encT_sb.append(t_)
            jw1_sb = []
            for k in range(4):
                t_ = singles.tile([128, J], F16, tag=f"jw1_{k}")
                nc.sync.dma_start(out=t_[:, :], in_=jw1enc[k * 128:(k + 1) * 128, :])
                jw1_sb.append(t_)
            jw2_sb = singles.tile([J, V], F16, tag="jw2")
            nc.sync.dma_start(out=jw2_sb[:, :], in_=jw2d[:, :])
            bias_sb = singles.tile([J, U1], F32, tag="biasT")
            nc.sync.dma_start(out=bias_sb[:, :], in_=biasT[:, :])
            ep_sb = singles.tile([128, TP], F16, tag="ep")

            for rep in range(reps):
                _emit_rep(nc, hidp, stgp, psp,
                          encT_sb, jw1_sb, jw2_sb, bias_sb, ep_sb, out, rep)

    nc.compile()
    return nc


def _emit_rep(nc, hidp, stgp, psp,
              encT_sb, jw1_sb, jw2_sb, bias_sb, ep_sb, out, rep):
    # ---- prologue: ep[j, t] = sum_e jw1enc[e, j] * encT[e, t] ----
    EC = 512
    for c in range(TP // EC):
        pe = psp.tile([128, EC], F32, tag="ps", name=f"ep{rep}_{c}")
        for k in range(4):
            nc.tensor.matmul(
                pe[:, :],
                jw1_sb[k][:, :],
                encT_sb[k][:, c * EC:(c + 1) * EC],
                start=(k == 0),
                stop=(k == 3),
            )
        nc.vector.tensor_copy(out=ep_sb[:, c * EC:(c + 1) * EC], in_=pe[:, :])

    # ---- u loop (t-major joint lattice) ----
    stg = None
    n_u = UD
    for u in range(U1):
        hid = hidp.tile([128, TP], F16, tag="hid", name=f"hid{rep}_{u}")
        nc.scalar.activation(
            out=hid[:, :],
            in_=ep_sb[:, :],
            func=mybir.ActivationFunctionType.Tanh,
            bias=bias_sb[:, u:u + 1],
            scale=1.0,
        )
        ps_a = psp.tile([128, TBA * V], F32, tag="ps", name=f"psa{rep}_{u}")
        ps_b = psp.tile([128, (NTB - TBA) * V], F32, tag="ps", name=f"psb{rep}_{u}")
        for tb in range(NTB):
            dst = (ps_a[:, tb * V:(tb + 1) * V] if tb < TBA
                   else ps_b[:, (tb - TBA) * V:(tb - TBA + 1) * V])
            nc.tensor.matmul(
                dst,
                hid[:, tb * TB:(tb + 1) * TB],
                jw2_sb[:, :],
                start=True,
                stop=True,
            )
        i = u % UD
        if i == 0:
            n_u = min(UD, U1 - u)
            stg = stgp.tile([128, NTB * n_u * V], F16, tag="stg",
                            name=f"stg{rep}_{u}")
        # stg col layout: tb * (n_u*V) + i * V + v
        stg_v = stg.rearrange("p (tb u v) -> p tb u v", tb=NTB, u=n_u)
        nc.vector.tensor_copy(
            out=stg_v[:, 0:TBA, i, :],
            in_=ps_a.rearrange("p (tb v) -> p tb v", tb=TBA),
        )
        nc.vector.tensor_copy(
            out=stg_v[:, TBA:NTB, i, :],
            in_=ps_b.rearrange("p (tb v) -> p tb v", tb=NTB - TBA),
        )
        if i == n_u - 1:
            u0 = u - i
            # DRAM rows t = tb*128 + p, cols [u0*V, (u0+n_u)*V)
            dst = out[:, u0 * V:(u0 + n_u) * V].rearrange(
                "(tb p) c -> p tb c", tb=NTB)
            nc.sync.dma_start(out=dst, in_=stg[:, :])


def _host_pred_bias(targets_b, emb, pw1, pb1, pw2, pb2, jw1, jb1):
    ext = np.concatenate([np.full(H, BLANK, np.int64), targets_b.astype(np.int64)])
    e = np.concatenate([emb[ext[1:U1 + 1]], emb[ext[0:U1]]], axis=1)
    h = np.tanh(e @ pw1 + pb1)
    pred = np.tanh(h @ pw2 + pb2)
    return (pred @ jw1[E:] + jb1).astype(np.float32)


def _make_in_maps(encoder_states, targets, emb, pw1, pb1, pw2, pb2, jw1, jb1, jw2):
    encoder_states = np.asarray(encoder_states, dtype=np.float32)
    jw1 = np.asarray(jw1, dtype=np.float32)
    jw2_np = np.ascontiguousarray(np.asarray(jw2, dtype=np.float32)).astype(np.float16)
    jw1enc = np.ascontiguousarray(jw1[:E]).astype(np.float16)

    in_maps = []
    for b in range(B):
        encT_b = np.zeros((E, TP), np.float16)
        encT_b[:, :T] = encoder_states[b].T.astype(np.float16)
        bias_b = _host_pred_bias(
            np.asarray(targets[b]), np.asarray(emb, np.float32),
            np.asarray(pw1, np.float32), np.asarray(pb1, np.float32),
            np.asarray(pw2, np.float32), np.asarray(pb2, np.float32),
            jw1, np.asarray(jb1, np.float32),
        )
        in_maps.append({
            "encT": encT_b,
            "jw1enc": jw1enc,
            "jw2d": jw2_np,
            "biasT": np.ascontiguousarray(bias_b.T),
        })
    return in_maps


def kernel(encoder_states, encoder_states_size, targets, targets_size,
           emb, pw1, pb1, pw2, pb2, jw1, jb1, jw2, jb2):
    if "nc" not in _CACHE:
        _CACHE["nc"] = _build_program()
    nc = _CACHE["nc"]

    in_maps = _make_in_maps(encoder_states, targets, emb, pw1, pb1, pw2, pb2,
                            jw1, jb1, jw2)
    _CACHE["in_maps"] = in_maps
    res = run_bass_kernel_spmd(nc, in_maps, core_ids=list(range(N_CORES)))

    out = np.empty((B, T, U1, V), np.float32)
    for b in range(B):
        out[b] = res.results[b]["out"][:T].astype(np.float32).reshape(T, U1, V)
    out += np.asarray(jb2, np.float32)
    tsz = np.asarray(encoder_states_size).astype(np.int64)
    usz = np.asarray(targets_size).astype(np.int64) + 1
    for b in range(B):
        out[b, tsz[b]:, :, :] = 0.0
        out[b, :, usz[b]:, :] = 0.0
    return out


# revision 7
# speedup vs baseline: 3.0101x; 3.0101x over previous
"""FFNN-Transducer joint-lattice kernel for 8 Trainium2 NeuronCores.

Data-parallel over batch B=8 (one sample per core). Per core the device
computes the dense T x (U+1) joint lattice:
    out[t,u,:] = tanh(enc_proj[t,:] + bias[u,:]) @ jw2
where bias[u,:] = (pred @ jw1[E:] + jb1)[u,:] comes from the tiny prediction
network (host, <0.3% of FLOPs); jb2-add, the [v,u,t]->[t,u,v] transpose and
ragged masking are host epilogues.

Device pipeline per core (all TensorE-facing tensors fp16, PSUM fp32):
  PE:   ep[j,t] = jw1enc.T @ encT (8 matmuls, K-accumulated);
        per u two joint matmuls ps[v, t-chunk] = jw2.T @ hid_u[:, chunk]
        (jw2 is the stationary operand for the WHOLE u loop; hid streams as
        the moving operand at 1 column/cycle -> ~1 PE cycle per lattice point).
  ACT:  per u one instruction hid_u = tanh(ep + bias_u) over [128, 1024],
        using the activation datapath's free per-partition bias operand.
  DVE:  per u one PSUM->SBUF evacuation [88, 1000] with fp32->fp16 cast.
  DMA:  one store per 2 u's, [88 rows x 4000B contiguous]; output DRAM layout
        is out[v, u*1000 + t] fp16 (host transposes).
"""

import os
import sys

for _p in ("/opt/trn_rl_repo", "/root/.axon_site/_ro/trn_rl_repo"):
    if os.path.isdir(_p) and _p not in sys.path:
        sys.path.append(_p)

import numpy as np

import concourse.bass as bass
import concourse.tile as tile
from concourse import bacc, mybir
from concourse.bass_utils import run_bass_kernel_spmd

# Problem dims (hardcoded per contract)
B, T, E = 8, 1000, 512
U = 100
U1 = U + 1          # 101 joint positions
H, D, P = 2, 256, 256
J, V = 128, 88
BLANK = V - 1
N_CORES = 8

# Device tiling
TP = 1024           # padded T (psum-bank-aligned matmul chunks)
TC = 512            # t columns per joint matmul / ep chunk (= one 2KB fp32 bank)
UD = 2              # u's per output DMA

F32 = mybir.dt.float32
F16 = mybir.dt.float16

_CACHE = {}


def _build_program(reps=1):
    nc = bacc.Bacc("TRN2", target_bir_lowering=False, debug=False)

    encT = nc.dram_tensor("encT", [E, TP], F16, kind="ExternalInput").ap()
    jw1enc = nc.dram_tensor("jw1enc", [E, J], F16, kind="ExternalInput").ap()
    jw2d = nc.dram_tensor("jw2d", [J, V], F16, kind="ExternalInput").ap()
    biasT = nc.dram_tensor("biasT", [J, U1], F32, kind="ExternalInput").ap()
    out = nc.dram_tensor("out", [V, U1 * T], F16, kind="ExternalOutput").ap()

    with tile.TileContext(nc) as tc:
        with (
            tc.tile_pool(name="singles", bufs=1) as singles,
            tc.tile_pool(name="hidp", bufs=3) as hidp,
            tc.tile_pool(name="stgp", bufs=3) as stgp,
            tc.tile_pool(name="psp", bufs=4, space="PSUM") as psp,
        ):
            # ---- persistent SBUF tensors ----
            encT_sb = []
            for k in range(4):
                t_ = singles.tile([128, TP], F16, tag=f"encT{k}")
                nc.sync.dma_start(out=t_[:, :], in_=encT[k * 128:(k + 1) * 128, :])
                encT_sb.append(t_)
            jw1_sb = []
            for k in range(4):
                t_ = singles.tile([128, J], F16, tag=f"jw1_{k}")
                nc.sync.dma_start(out=t_[:, :], in_=jw1enc[k * 128:(k + 1) * 128, :])
                jw1_sb.append(t_)
            jw2_sb = singles.tile([J, V], F16, tag="jw2")
            nc.sync.dma_start(out=jw2_sb[:, :], in_=jw2d[:, :])
            bias_sb = singles.tile([J, U1], F32, tag="biasT")
            nc.sync.dma_start(out=bias_sb[:, :], in_=biasT[:, :])
            # enc_projT[j, t], fp16, lives in SBUF across the whole u loop
            ep_sb = singles.tile([128, TP], F16, tag="ep")

            for rep in range(reps):
                _emit_rep(nc, hidp, stgp, psp,
                          encT_sb, jw1_sb, jw2_sb, bias_sb, ep_sb, out, rep)

    nc.compile()
    return nc


def _emit_rep(nc, hidp, stgp, psp,
              encT_sb, jw1_sb, jw2_sb, bias_sb, ep_sb, out, rep):
    # ---- prologue: ep[j, t] = sum_e jw1enc[e, j] * encT[e, t] ----
    for c in range(TP // TC):
        pe = psp.tile([128, TC], F32, tag="ps", name=f"ep{rep}_{c}")
        for k in range(4):
            nc.tensor.matmul(
                pe[:, :],
                jw1_sb[k][:, :],
                encT_sb[k][:, c * TC:(c + 1) * TC],
                start=(k == 0),
                stop=(k == 3),
            )
        nc.vector.tensor_copy(out=ep_sb[:, c * TC:(c + 1) * TC], in_=pe[:, :])

    # ---- u loop (v-major joint lattice) ----
    stg = None
    for u in range(U1):
        hid = hidp.tile([128, TP], F16, tag="hid", name=f"hid{rep}_{u}")
        nc.scalar.activation(
            out=hid[:, :],
            in_=ep_sb[:, :],
            func=mybir.ActivationFunctionType.Tanh,
            bias=bias_sb[:, u:u + 1],
            scale=1.0,
        )
        ps = psp.tile([V, TP], F32, tag="ps", name=f"ps{rep}_{u}")
        for c in range(TP // TC):
            nc.tensor.matmul(
                ps[:, c * TC:(c + 1) * TC],
                jw2_sb[:, :],
                hid[:, c * TC:(c + 1) * TC],
                start=True,
                stop=True,
            )
        i = u % UD
        if i == 0:
            n_u = min(UD, U1 - u)
            stg = stgp.tile([V, n_u * T], F16, tag="stg", name=f"stg{rep}_{u}")
        nc.vector.tensor_copy(out=stg[:, i * T:(i + 1) * T], in_=ps[:, 0:T])
        if i == n_u - 1:
            u0 = u - i
            nc.sync.dma_start(
                out=out[:, u0 * T:(u0 + n_u) * T],
                in_=stg[:, :],
            )


def _host_pred_bias(targets_b, emb, pw1, pb1, pw2, pb2, jw1, jb1):
    """bias[u, j] = (pred @ jw1[E:] + jb1)[u, j] for the 101 joint positions."""
    ext = np.concatenate([np.full(H, BLANK, np.int64), targets_b.astype(np.int64)])
    e = np.concatenate([emb[ext[1:U1 + 1]], emb[ext[0:U1]]], axis=1)  # [101, 512]
    h = np.tanh(e @ pw1 + pb1)
    pred = np.tanh(h @ pw2 + pb2)
    return (pred @ jw1[E:] + jb1).astype(np.float32)  # [101, 128]


def _make_in_maps(encoder_states, targets, emb, pw1, pb1, pw2, pb2, jw1, jb1, jw2):
    encoder_states = np.asarray(encoder_states, dtype=np.float32)
    jw1 = np.asarray(jw1, dtype=np.float32)
    jw2_np = np.ascontiguousarray(np.asarray(jw2, dtype=np.float32)).astype(np.float16)
    jw1enc = np.ascontiguousarray(jw1[:E]).astype(np.float16)

    in_maps = []
    for b in range(B):
        encT_b = np.zeros((E, TP), np.float16)
        encT_b[:, :T] = encoder_states[b].T.astype(np.float16)
        bias_b = _host_pred_bias(
            np.asarray(targets[b]), np.asarray(emb, np.float32),
            np.asarray(pw1, np.float32), np.asarray(pb1, np.float32),
            np.asarray(pw2, np.float32), np.asarray(pb2, np.float32),
            jw1, np.asarray(jb1, np.float32),
        )  # [101, 128] f32
        in_maps.append({
            "encT": encT_b,
            "jw1enc": jw1enc,
            "jw2d": jw2_np,
            "biasT": np.ascontiguousarray(bias_b.T),  # [128, 101] f32
        })
    return in_maps


def kernel(encoder_states, encoder_states_size, targets, targets_size,
           emb, pw1, pb1, pw2, pb2, jw1, jb1, jw2, jb2):
    if "nc" not in _CACHE:
        _CACHE["nc"] = _build_program()
    nc = _CACHE["nc"]

    in_maps = _make_in_maps(encoder_states, targets, emb, pw1, pb1, pw2, pb2,
                            jw1, jb1, jw2)
    _CACHE["in_maps"] = in_maps
    res = run_bass_kernel_spmd(nc, in_maps, core_ids=list(range(N_CORES)))

    out = np.empty((B, T, U1, V), np.float32)
    for b in range(B):
        o = res.results[b]["out"].reshape(V, U1, T)   # out[v, u, t] fp16
        out[b] = o.transpose(2, 1, 0).astype(np.float32)
    out += np.asarray(jb2, np.float32)  # jb2 epilogue (host)
    # ragged masking (host epilogue)
    tsz = np.asarray(encoder_states_size).astype(np.int64)
    usz = np.asarray(targets_size).astype(np.int64) + 1
    for b in range(B):
        out[b, tsz[b]:, :, :] = 0.0
        out[b, :, usz[b]:, :] = 0.0
    return out


# revision 10
# speedup vs baseline: 3.3071x; 1.0987x over previous
"""FFNN-Transducer joint-lattice kernel for 8 Trainium2 NeuronCores.

Data-parallel over batch B=8 (one sample per core). Per core the device
computes the dense T x (U+1) joint lattice:
    out[t,u,:] = tanh(enc_proj[t,:] + bias[u,:]) @ jw2
where bias[u,:] = (pred @ jw1[E:] + jb1)[u,:] comes from the tiny prediction
network (host, <0.3% of FLOPs); jb2-add, the [v,u,t]->[t,u,v] transpose and
ragged masking are host epilogues.

Device pipeline per core (all TensorE-facing tensors fp16, PSUM fp32):
  PE:   ep[j,t] = jw1enc.T @ encT (8 matmuls, K-accumulated);
        per u two joint matmuls ps[v, t-chunk] = jw2.T @ hid_u[:, chunk]
        (jw2 is the stationary operand for the WHOLE u loop; hid streams as
        the moving operand at 1 column/cycle -> ~1 PE cycle per lattice point).
  ACT:  per u one instruction hid_u = tanh(ep + bias_u) over [128, 1024],
        using the activation datapath's free per-partition bias operand.
  DVE:  per u one PSUM->SBUF evacuation [88, 1000] with fp32->fp16 cast.
  DMA:  one store per 2 u's, [88 rows x 4000B contiguous]; output DRAM layout
        is out[v, u*1000 + t] fp16 (host transposes).
"""

import os
import sys

for _p in ("/opt/trn_rl_repo", "/root/.axon_site/_ro/trn_rl_repo"):
    if os.path.isdir(_p) and _p not in sys.path:
        sys.path.append(_p)

import numpy as np

import concourse.bass as bass
import concourse.tile as tile
from concourse import bacc, mybir
from concourse.bass_utils import run_bass_kernel_spmd

# Problem dims (hardcoded per contract)
B, T, E = 8, 1000, 512
U = 100
U1 = U + 1          # 101 joint positions
H, D, P = 2, 256, 256
J, V = 128, 88
BLANK = V - 1
N_CORES = 8

# Device tiling
TP = 1024           # padded T (psum-bank-aligned matmul chunks)
TC = 512            # t columns per joint matmul / ep chunk (= one 2KB fp32 bank)
UD = 8              # u's per output DMA

F32 = mybir.dt.float32
F16 = mybir.dt.float16

_CACHE = {}


def _build_program(reps=1):
    nc = bacc.Bacc("TRN2", target_bir_lowering=False, debug=False)

    encT = nc.dram_tensor("encT", [E, TP], F16, kind="ExternalInput").ap()
    jw1enc = nc.dram_tensor("jw1enc", [E, J], F16, kind="ExternalInput").ap()
    jw2d = nc.dram_tensor("jw2d", [J, V], F16, kind="ExternalInput").ap()
    biasT = nc.dram_tensor("biasT", [J, U1], F32, kind="ExternalInput").ap()
    out = nc.dram_tensor("out", [V, U1 * T], F16, kind="ExternalOutput").ap()

    with tile.TileContext(nc) as tc:
        with (
            tc.tile_pool(name="singles", bufs=1) as singles,
            tc.tile_pool(name="hidp", bufs=2) as hidp,
            tc.tile_pool(name="stgp", bufs=3) as stgp,
            tc.tile_pool(name="psp", bufs=2, space="PSUM") as psp,
        ):
            # ---- persistent SBUF tensors ----
            encT_sb = []
            for k in range(4):
                t_ = singles.tile([128, TP], F16, tag=f"encT{k}")
                nc.sync.dma_start(out=t_[:, :], in_=encT[k * 128:(k + 1) * 128, :])
                encT_sb.append(t_)
            jw1_sb = []
            for k in range(4):
                t_ = singles.tile([128, J], F16, tag=f"jw1_{k}")
                nc.sync.dma_start(out=t_[:, :], in_=jw1enc[k * 128:(k + 1) * 128, :])
                jw1_sb.append(t_)
            jw2_sb = singles.tile([J, V], F16, tag="jw2")
            nc.sync.dma_start(out=jw2_sb[:, :], in_=jw2d[:, :])
            bias_sb = singles.tile([J, U1], F32, tag="biasT")
            nc.sync.dma_start(out=bias_sb[:, :], in_=biasT[:, :])
            # enc_projT[j, t], fp16, lives in SBUF across the whole u loop
            ep_sb = singles.tile([128, TP], F16, tag="ep")

            for rep in range(reps):
                _emit_rep(nc, hidp, stgp, psp,
                          encT_sb, jw1_sb, jw2_sb, bias_sb, ep_sb, out, rep)

    nc.compile()
    _strip_redundant_ldweights(nc)
    return nc


def _strip_redundant_ldweights(nc):
    """Delete InstLdweights whose weights AP matches the stationary operand
    already loaded by the previous kept load in the same block; move any
    semaphore waits they carry onto the following PE instruction."""
    import bass_rust
    for bb in nc.m.functions[0].blocks:
        insts = bb.instructions
        last_sig = None
        drop = set()
        pending = []   # waits to reattach to the next PE instruction
        for idx, inst in enumerate(insts):
            if inst.engine != mybir.EngineType.PE:
                continue
            name = type(inst).__name__
            if name == "InstLdweights":
                sig = (str(inst.ins[0]), str(inst.perf_mode),
                       str(inst.is_transpose))
                has_sync = (inst.sync_info is not None
                            and (len(inst.sync_info.on_update) > 0
                                 or len(inst.sync_info.on_wait) > 0))
                if sig == last_sig and not has_sync:
                    drop.add(idx)
                else:
                    last_sig = sig
        if drop:
            bb.instructions = [i for k, i in enumerate(insts)
                               if k not in drop]


def _emit_rep(nc, hidp, stgp, psp,
              encT_sb, jw1_sb, jw2_sb, bias_sb, ep_sb, out, rep):
    # ---- prologue: ep[j, t] = sum_e jw1enc[e, j] * encT[e, t] ----
    for c in range(TP // TC):
        pe = psp.tile([128, TC], F32, tag="ps", name=f"ep{rep}_{c}")
        for k in range(4):
            nc.tensor.matmul(
                pe[:, :],
                jw1_sb[k][:, :],
                encT_sb[k][:, c * TC:(c + 1) * TC],
                start=(k == 0),
                stop=(k == 3),
            )
        nc.vector.tensor_copy(out=ep_sb[:, c * TC:(c + 1) * TC], in_=pe[:, :])

    # ---- u loop (v-major joint lattice), u's processed in pairs ----
    stg = None
    n_u = UD
    for p in range((U1 + 1) // 2):
        u0 = 2 * p
        n_p = min(2, U1 - u0)          # 2, except the last lone u
        hid2 = hidp.tile([128, 2 * TP], F16, tag="hid", name=f"hid{rep}_{p}")
        ps2 = psp.tile([V, 2 * TP], F32, tag="ps", name=f"ps{rep}_{p}")
        for i in range(n_p):
            u = u0 + i
            nc.scalar.activation(
                out=hid2[:, i * TP:(i + 1) * TP],
                in_=ep_sb[:, :],
                func=mybir.ActivationFunctionType.Tanh,
                bias=bias_sb[:, u:u + 1],
                scale=1.0,
            )
            for c in range(TP // TC):
                nc.tensor.matmul(
                    ps2[:, i * TP + c * TC:i * TP + (c + 1) * TC],
                    jw2_sb[:, :],
                    hid2[:, i * TP + c * TC:i * TP + (c + 1) * TC],
                    start=True,
                    stop=True,
                )
        j = u0 % UD
        if j == 0:
            n_u = min(UD, U1 - u0)
            stg = stgp.tile([V, n_u * T], F16, tag="stg", name=f"stg{rep}_{u0}")
        # evacuate n_p u's in one strided copy [V, n_p, T]
        nc.vector.tensor_copy(
            out=stg.rearrange("v (u t) -> v u t", t=T)[:, j:j + n_p, :],
            in_=ps2.rearrange("v (u t) -> v u t", t=TP)[:, 0:n_p, 0:T],
        )
        if j + n_p >= n_u:
            nc.sync.dma_start(
                out=out[:, (u0 - j) * T:(u0 - j + n_u) * T],
                in_=stg[:, 0:n_u * T],
            )


def _host_pred_bias(targets_b, emb, pw1, pb1, pw2, pb2, jw1, jb1):
    """bias[u, j] = (pred @ jw1[E:] + jb1)[u, j] for the 101 joint positions."""
    ext = np.concatenate([np.full(H, BLANK, np.int64), targets_b.astype(np.int64)])
    e = np.concatenate([emb[ext[1:U1 + 1]], emb[ext[0:U1]]], axis=1)  # [101, 512]
    h = np.tanh(e @ pw1 + pb1)
    pred = np.tanh(h @ pw2 + pb2)
    return (pred @ jw1[E:] + jb1).astype(np.float32)  # [101, 128]


def _make_in_maps(encoder_states, targets, emb, pw1, pb1, pw2, pb2, jw1, jb1, jw2):
    encoder_states = np.asarray(encoder_states, dtype=np.float32)
    jw1 = np.asarray(jw1, dtype=np.float32)
    jw2_np = np.ascontiguousarray(np.asarray(jw2, dtype=np.float32)).astype(np.float16)
    jw1enc = np.ascontiguousarray(jw1[:E]).astype(np.float16)

    in_maps = []
    for b in range(B):
        encT_b = np.zeros((E, TP), np.float16)
        encT_b[:, :T] = encoder_states[b].T.astype(np.float16)
        bias_b = _host_pred_bias(
            np.asarray(targets[b]), np.asarray(emb, np.float32),
            np.asarray(pw1, np.float32), np.asarray(pb1, np.float32),
            np.asarray(pw2, np.float32), np.asarray(pb2, np.float32),
            jw1, np.asarray(jb1, np.float32),
        )  # [101, 128] f32
        in_maps.append({
            "encT": encT_b,
            "jw1enc": jw1enc,
            "jw2d": jw2_np,
            "biasT": np.ascontiguousarray(bias_b.T),  # [128, 101] f32
        })
    return in_maps


def kernel(encoder_states, encoder_states_size, targets, targets_size,
           emb, pw1, pb1, pw2, pb2, jw1, jb1, jw2, jb2):
    if "nc" not in _CACHE:
        _CACHE["nc"] = _build_program()
    nc = _CACHE["nc"]

    in_maps = _make_in_maps(encoder_states, targets, emb, pw1, pb1, pw2, pb2,
                            jw1, jb1, jw2)
    _CACHE["in_maps"] = in_maps
    res = run_bass_kernel_spmd(nc, in_maps, core_ids=list(range(N_CORES)))

    out = np.empty((B, T, U1, V), np.float32)
    for b in range(B):
        o = res.results[b]["out"].reshape(V, U1, T)   # out[v, u, t] fp16
        out[b] = o.transpose(2, 1, 0).astype(np.float32)
    out += np.asarray(jb2, np.float32)  # jb2 epilogue (host)
    # ragged masking (host epilogue)
    tsz = np.asarray(encoder_states_size).astype(np.int64)
    usz = np.asarray(targets_size).astype(np.int64) + 1
    for b in range(B):
        out[b, tsz[b]:, :, :] = 0.0
        out[b, :, usz[b]:, :] = 0.0
    return out
